# revision 1
# baseline (speedup 1.0000x reference)
"""MinGRU cell on 8 Trainium2 NeuronCores (Bass/Tile).

Math (per batch b, hidden h):
    gz = x @ W_z^T ; gh = x @ W_h^T                 (two GEMMs, K=D=1024)
    z  = sigmoid(gz + b_z)
    h_t = (1 - z_t) * h_{t-1} + z_t * (gh_t + b_h)  (affine scan over T)

Distribution: data-parallel over batch B=16 -> 2 batches per core, weights
replicated; no cross-core communication.

Per-core pipeline (software-pipelined over 8 steps of 512 tokens):
  x [t,d] --SWDGE cast-load--> bf16 --PE transpose--> xT [d,t]
  GEMMs with W^T stationary (bf16, fp32 PSUM accumulation), output [h, t]
  ACT: a = sigmoid(-gz - b_z) = 1-z ; z = sigmoid(gz + b_z)
  DVE: bsc = (gh + b_h) * z ; h = tensor_tensor_scan(a, bsc) along t
  PE transpose h back to [t, h], DMA out.

The PE stream is in-order, so emission order is chosen to avoid
head-of-line blocking: per step s we emit GEMMs(s), then the
out-transposes of step s-1 (whose scans finished long ago), then the
x-transposes for step s+1.
"""

import sys

sys.path.insert(0, "/opt/trn_rl_repo")

from contextlib import ExitStack

import numpy as np

import concourse.bass as bass
import concourse.mybir as mybir
import concourse.tile as tile
from concourse import bacc
from concourse.bass import ts, ds
from concourse.bass_utils import run_bass_kernel_spmd
from concourse.masks import make_identity

B, T, D, H = 16, 2048, 1024, 1024
NCORES = 8
B_LOC = B // NCORES  # 2
P = 128
TC = 512  # tokens per step
NSTEP = B_LOC * T // TC  # 8
NTC = T // TC  # 4 steps per batch
TSUB = TC // P  # 4
DC = D // P  # 8 contraction chunks
HC = H // P  # 8 hidden chunks

F32 = mybir.dt.float32
BF16 = mybir.dt.bfloat16
AF = mybir.ActivationFunctionType
OP = mybir.AluOpType

_CACHE = {}


class _State:
    pass


def _mingru_tile(tc, out, x, h0, wz, bz, wh, bh):
    nc = tc.nc
    st = _State()

    with ExitStack() as ctx:
        consts = ctx.enter_context(tc.tile_pool(name="consts", bufs=1))

        id_bf = consts.tile([P, P], BF16)
        make_identity(nc, id_bf)
        id_f32 = consts.tile([P, P], F32)
        make_identity(nc, id_f32)

        bz_sb = consts.tile([P, HC], F32)
        nc.sync.dma_start(out=bz_sb, in_=bz.rearrange("(c p) -> p c", p=P))
        bh_sb = consts.tile([P, HC], F32)
        nc.sync.dma_start(out=bh_sb, in_=bh.rearrange("(c p) -> p c", p=P))
        nbz_sb = consts.tile([P, HC], F32)
        nc.vector.tensor_scalar_mul(nbz_sb, bz_sb, -1.0)
        hp_sb = consts.tile([P, B_LOC * HC], F32)
        nc.sync.dma_start(out=hp_sb, in_=h0.rearrange("b (c p) -> p (b c)", p=P))

        # xnat is the only pool needed before the W build; the rest are
        # opened after it so the W-build PSUM pool can use all 8 banks.
        xnat_p = ctx.enter_context(tc.tile_pool(name="xnat", bufs=2))
        xt_p = azb_p = scan_p = onat_p = pz_p = ph_p = pxt_p = po_p = None

        st.xn = {}  # step -> [4 natural x tiles]
        st.xt = {}  # step -> [8 xT tiles]
        st.scan = {}  # step -> [8 scan tiles]

        def step_bt(s):
            return s // NTC, s % NTC

        def sect_A(s):  # x cast-loads (SWDGE)
            b, tci = step_bt(s)
            tiles = []
            for j in range(TSUB):
                xt_nat = xnat_p.tile([P, D], BF16, tag=f"xn{j}", name=f"xn_{s}_{j}")
                nc.gpsimd.dma_start(
                    out=xt_nat, in_=x[b, ds(tci * TC + j * P, P), :]
                )
                tiles.append(xt_nat)
            st.xn[s] = tiles

        def sect_B(s):  # x transposes (PE) + copies (ACT)
            xn = st.xn.pop(s)
            tiles = []
            for dc in range(DC):
                pxt = pxt_p.tile([P, TC], BF16, tag="pxt", name=f"pxt_{s}_{dc}")
                for j in range(TSUB):
                    nc.tensor.transpose(pxt[:, ts(j, P)], xn[j][:, ts(dc, P)], id_bf)
                xt_sb = xt_p.tile([P, TC], BF16, tag=f"xt{dc}", name=f"xt_{s}_{dc}")
                nc.scalar.copy(xt_sb, pxt)
                tiles.append(xt_sb)
            st.xt[s] = tiles

        HH = H // 2

        def gemm(s, hc, which):
            xts = st.xt[s]
            wt = st.wt[which][hc // 4]
            pool = pz_p if which == "z" else ph_p
            psum = pool.tile(
                [P, TC], F32, tag="pz" if which == "z" else "ph",
                name=f"ps{which}_{s}_{hc}",
            )
            for dc in range(DC):
                nc.tensor.matmul(
                    psum,
                    wt[:, ds(dc * HH + (hc % 4) * P, P)],
                    xts[dc],
                    start=(dc == 0),
                    stop=(dc == DC - 1),
                )
            return psum

        def sect_post(s, hc, psum_z, psum_h):
            b, tci = step_bt(s)
            a_sb = azb_p.tile([P, TC], F32, tag="a", name=f"a_{s}_{hc}")
            nc.scalar.activation(
                a_sb, psum_z, AF.Sigmoid, bias=nbz_sb[:, hc : hc + 1], scale=-1.0
            )
            z_sb = azb_p.tile([P, TC], F32, tag="z", name=f"z_{s}_{hc}")
            nc.scalar.activation(
                z_sb, psum_z, AF.Sigmoid, bias=bz_sb[:, hc : hc + 1], scale=1.0
            )
            bsc = azb_p.tile([P, TC], F32, tag="b", name=f"b_{s}_{hc}")
            nc.vector.scalar_tensor_tensor(
                bsc, psum_h, bh_sb[:, hc : hc + 1], z_sb, op0=OP.add, op1=OP.mult
            )
            # bf16 scan output: the scan's accumulator state is fp32 in HW
            # regardless of out dtype, so only stored values round (~2^-9).
            # This makes the out-transposes bf16 (1 cyc/row vs 2 on PE).
            sc = scan_p.tile([P, TC], BF16, tag=f"sc{hc}", name=f"sc_{s}_{hc}")
            if tci == 0:
                init = hp_sb[:, b * HC + hc : b * HC + hc + 1]
            else:
                init = st.scan[s - 1][hc][:, TC - 1 : TC]
            nc.vector.tensor_tensor_scan(sc, a_sb, bsc, init, op0=OP.mult, op1=OP.add)
            st.scan.setdefault(s, [None] * HC)[hc] = sc

        def sect_CD(s, z_first=False, mid_hook=None):
            if z_first:
                # step 0: W_h^T lands after W_z^T; run all z GEMMs first
                pzs = [gemm(s, hc, "z") for hc in range(HC)]
                phs = [gemm(s, hc, "h") for hc in range(HC)]
                for hc in range(HC):
                    sect_post(s, hc, pzs[hc], phs[hc])
                if mid_hook is not None:
                    mid_hook()
            else:
                for hc in range(HC):
                    psum_z = gemm(s, hc, "z")
                    psum_h = gemm(s, hc, "h")
                    sect_post(s, hc, psum_z, psum_h)
                    if hc == 3 and mid_hook is not None:
                        mid_hook()

        def sect_E(s):  # out transposes (PE) + copies (DVE) + store
            b, tci = step_bt(s)
            scans = st.scan[s]
            # Last two steps: DVE is saturated by the final scan chain,
            # ACT is idle — put the tail copies there.
            last = s >= NSTEP - 2
            for j in range(TSUB):
                on = onat_p.tile([P, H], F32, tag="on", name=f"on_{s}_{j}")
                for half in range(2):
                    po = po_p.tile(
                        [P, TC], BF16, tag="po", name=f"po_{s}_{j}_{half}"
                    )
                    for k in range(4):
                        hc = half * 4 + k
                        nc.tensor.transpose(
                            po[:, ts(k, P)], scans[hc][:, ts(j, P)], id_bf
                        )
                    if last:
                        nc.scalar.copy(on[:, ds(half * TC, TC)], po)
                    else:
                        nc.vector.tensor_copy(on[:, ds(half * TC, TC)], po)
                nc.sync.dma_start(out=out[b, ds(tci * TC + j * P, P), :], in_=on)
            if s - 1 in st.scan:
                del st.scan[s - 1]

        # --- prologue ---------------------------------------------------
        sect_A(0)
        sect_A(1)

        # HAM warmup: ~4.3us of back-to-back junk matmuls so the PE clock
        # gate opens (K=8/8) while the W DMA streams in; the first real
        # GEMMs then run at 2.4 GHz instead of 1.2. The DMA to a DRAM
        # scratch keeps the chain from being DCE'd.
        with tc.tile_pool(name="warm", bufs=1, space="PSUM") as warm_p, \
             tc.tile_pool(name="wdram", bufs=1, space="DRAM") as wdram_p:
            junk_ps = warm_p.tile([P, P], F32, name="junk_ps")
            NWARM = 40
            for i in range(NWARM):
                nc.tensor.matmul(
                    junk_ps, id_bf, id_bf, start=(i == 0), stop=(i == NWARM - 1)
                )
            junk_sb = consts.tile([P, P], F32, name="junk_sb")
            nc.vector.tensor_copy(junk_sb, junk_ps)
            junk_dr = wdram_p.tile([P, P], F32, name="junk_dr")
            nc.sync.dma_start(out=junk_dr, in_=junk_sb)

        # W^T build, staged in h-halves. Each half is its own SBUF tile so
        # GEMMs reading it are gated only on that half's 8 copies (Tile
        # deps are tile-granular) — the first GEMMs start as soon as the
        # first 2 MB of W has landed, not after all 8 MB.
        # wt[r][:, dc*(H/2) + (hc%4)*P + hp] = W[hc*P + hp, dc*P + dp]
        HHALF = 4 * P
        st.wt = {
            "z": [
                consts.tile([P, DC * HHALF], BF16, name="wt_z0"),
                consts.tile([P, DC * HHALF], BF16, name="wt_z1"),
            ],
            "h": [
                consts.tile([P, DC * HHALF], BF16, name="wt_h0"),
                consts.tile([P, DC * HHALF], BF16, name="wt_h1"),
            ],
        }
        with tc.tile_pool(name="wnat", bufs=1) as wnat_p, tc.tile_pool(
            name="pwt", bufs=1, space="PSUM"
        ) as pwt_p:
            for w_ap, wn in ((wz, "z"), (wh, "h")):
                for r in range(2):  # h-halves
                    wt_sb = st.wt[wn][r]
                    wnat = []
                    for k in range(4):
                        hc = r * 4 + k
                        t_ = wnat_p.tile(
                            [P, D], F32, tag=f"wn{k}", name=f"wn_{wn}_{hc}"
                        )
                        nc.sync.dma_start(out=t_, in_=w_ap[ts(hc, P), :])
                        wnat.append(t_)
                    pws = []
                    for dc in range(DC):
                        pw = pwt_p.tile(
                            [P, TC], F32, tag=f"pw{dc}", name=f"pw_{wn}_{r}_{dc}"
                        )
                        for k in range(4):
                            nc.tensor.transpose(
                                pw[:, ts(k, P)], wnat[k][:, ts(dc, P)], id_f32
                            )
                        pws.append(pw)
                    for dc in range(DC):
                        nc.scalar.copy(
                            wt_sb[:, ds(dc * HHALF, 4 * P)], pws[dc]
                        )

        # --- remaining pools (after W-build PSUM pool released) ---------
        xt_p = ctx.enter_context(tc.tile_pool(name="xt", bufs=2))
        azb_p = ctx.enter_context(tc.tile_pool(name="azb", bufs=2))
        scan_p = ctx.enter_context(tc.tile_pool(name="scan", bufs=2))
        onat_p = ctx.enter_context(tc.tile_pool(name="onat", bufs=3))
        # PSUM: pz(2) + ph(2) + pxt(2) + po(2) = 8 banks
        pz_p = ctx.enter_context(tc.tile_pool(name="pz", bufs=2, space="PSUM"))
        ph_p = ctx.enter_context(tc.tile_pool(name="ph", bufs=2, space="PSUM"))
        pxt_p = ctx.enter_context(tc.tile_pool(name="pxt", bufs=2, space="PSUM"))
        po_p = ctx.enter_context(tc.tile_pool(name="po", bufs=2, space="PSUM"))

        sect_B(0)
        sect_B(1)

        # --- steady state ----------------------------------------------
        # B(s+1) is emitted mid-GEMM (after hc=3) so its PSUM staging and
        # ACT copies complete well before step s ends; the boundary then
        # has GEMM(s+1) ready immediately.
        for s in range(NSTEP):
            if s >= 1 and s + 1 < NSTEP:
                sect_A(s + 1)
            hook = (lambda s=s: sect_B(s + 1)) if (s >= 1 and s + 1 < NSTEP) else None
            sect_CD(s, z_first=(s == 0), mid_hook=hook)
            if s >= 1:
                sect_E(s - 1)
        sect_E(NSTEP - 1)


def build():
    if "nc" in _CACHE:
        return _CACHE["nc"]
    nc = bacc.Bacc(
        "TRN2", target_bir_lowering=False, debug=False, num_devices=NCORES
    )
    x = nc.dram_tensor("x", [B_LOC, T, D], F32, kind="ExternalInput").ap()
    h0 = nc.dram_tensor("h0", [B_LOC, H], F32, kind="ExternalInput").ap()
    wz = nc.dram_tensor("wz", [H, D], F32, kind="ExternalInput").ap()
    bz = nc.dram_tensor("bz", [H], F32, kind="ExternalInput").ap()
    wh = nc.dram_tensor("wh", [H, D], F32, kind="ExternalInput").ap()
    bh = nc.dram_tensor("bh", [H], F32, kind="ExternalInput").ap()
    out = nc.dram_tensor("out", [B_LOC, T, H], F32, kind="ExternalOutput").ap()
    with tile.TileContext(nc) as tctx:
        _mingru_tile(tctx, out, x, h0, wz, bz, wh, bh)
    nc.compile()
    _CACHE["nc"] = nc
    return nc


def make_in_maps(x, h_prev, W_z, b_z, W_h, b_h):
    x = np.ascontiguousarray(np.asarray(x, dtype=np.float32))
    h_prev = np.ascontiguousarray(np.asarray(h_prev, dtype=np.float32))
    W_z = np.ascontiguousarray(np.asarray(W_z, dtype=np.float32))
    b_z = np.ascontiguousarray(np.asarray(b_z, dtype=np.float32))
    W_h = np.ascontiguousarray(np.asarray(W_h, dtype=np.float32))
    b_h = np.ascontiguousarray(np.asarray(b_h, dtype=np.float32))
    in_maps = []
    for c in range(NCORES):
        sl = slice(c * B_LOC, (c + 1) * B_LOC)
        in_maps.append(
            {
                "x": x[sl],
                "h0": h_prev[sl],
                "wz": W_z,
                "bz": b_z,
                "wh": W_h,
                "bh": b_h,
            }
        )
    return in_maps


def kernel(x, h_prev, W_z, b_z, W_h, b_h, trace=False):
    nc = build()
    in_maps = make_in_maps(x, h_prev, W_z, b_z, W_h, b_h)
    res = run_bass_kernel_spmd(
        nc, in_maps, core_ids=list(range(NCORES)), trace=trace
    )
    out = np.concatenate([r["out"] for r in res.results], axis=0)
    if trace:
        _CACHE["last_results"] = res
    return out

